# revision 22
# baseline (speedup 1.0000x reference)
"""AltAttention (B=2,S=2048,D=1024,H=16, ALiBi + key-mask) on 8 TRN2 cores.

Sharding: core c = (b = c//4, head-group g = c%4 -> heads {g, g+4, g+8, g+12}).
Each core computes QKV for its 4 heads, attention, and a partial output
projection (row-split Wproj).  Host sums the 4 partials per batch and adds
bproj.

On-chip layout is fully transposed: scores S^T=[k,q], context ctx^T=[dh,q],
output out^T=[dout,q].  All matmuls bf16 (fp32 PSUM accumulate), attention
and projection use N=1024 moving operands.

ALiBi:
 - banding: tiles where exp(-slope|k-q|) < e^-15 everywhere are skipped.
   Band profile per local head slot = max cutoff over the 4 interleaved
   head sets, so the graph is SPMD-identical across cores.
 - local heads 0,1 (steep): P = exp(S+mask) * E, E = exp(-slope|k-q|)
   host-precomputed per diagonal offset (bf16, deduplicated).
 - local heads 2,3 (shallow): exp(-slope|k-q|) = [u(k)*v(q)] * R where
   u(k)=e^{slope(k-1024)} is folded into a scaled copy of [V|1], v(q)
   cancels in softmax normalization, and R = 1 for k<=q,
   exp(-2 slope (k-q)) for k>q is a (sparser) E-table multiply.
"""

import sys

for _p in ("/opt/trn_rl_repo", "/opt/pypackages"):
    if _p not in sys.path:
        sys.path.insert(0, _p)

import numpy as np
import ml_dtypes

import concourse.bass as bass
from concourse import bacc
import concourse.mybir as mybir
import concourse.tile as tile
from concourse.bass_utils import run_bass_kernel_spmd

BF16 = ml_dtypes.bfloat16

B, S, D, H = 2, 2048, 1024, 16
DH = D // H
HPC = 4
SCALE = D ** -0.5
NKT = S // 128       # 16
NW = S // 1024       # 2 q-windows of 1024
NDT = D // 128       # 8
CENT = 1024

CUTS = [60, 240, 960, 99999]


def _band(hl, w):
    cut = CUTS[hl]
    q0 = w * 1024
    lo, hi = max(0, q0 - cut), min(S, q0 + 1024 + cut)
    return [kt for kt in range(NKT) if kt * 128 < hi and (kt + 1) * 128 > lo]


BANDS = [[_band(hl, w) for w in range(NW)] for hl in range(4)]

# E multiply works at [128,512] half-window granularity.
# delta = kt*128 - half*512 over all (kt, half) pairs of the banded windows.
EDELT = {}
for hl in range(4):
    ds = set()
    for w in range(NW):
        for kt in BANDS[hl][w]:
            for half in (2 * w, 2 * w + 1):
                dlt = kt * 128 - half * 512
                if hl < 2 or dlt > -128:
                    ds.add(dlt)
    EDELT[hl] = sorted(ds)
EIDX = {hl: {d: i for i, d in enumerate(EDELT[hl])} for hl in range(4)}
ESLOT = [len(EDELT[hl]) for hl in range(4)]
EOFF = [0, ESLOT[0], ESLOT[0] + ESLOT[1], ESLOT[0] + ESLOT[1] + ESLOT[2]]
ETOT = sum(ESLOT)

_F32 = mybir.dt.float32
_BF = mybir.dt.bfloat16


def build_bass():
    nc = bacc.Bacc(None, target_bir_lowering=False)
    xt = nc.declare_dram_parameter("xt", [D, S], _BF, isOutput=False)
    wqk = nc.declare_dram_parameter("wqk", [D, 2 * HPC * DH], _BF, isOutput=False)
    wqkb = nc.declare_dram_parameter("wqkb", [1, 2 * HPC * DH], _BF, isOutput=False)
    wv = nc.declare_dram_parameter("wv", [D, HPC * DH], _BF, isOutput=False)
    wvb = nc.declare_dram_parameter("wvb", [1, HPC * DH], _BF, isOutput=False)
    wp = nc.declare_dram_parameter("wp", [HPC * DH, D], _BF, isOutput=False)
    etab = nc.declare_dram_parameter("etab", [128, ETOT * 512], _BF, isOutput=False)
    utab = nc.declare_dram_parameter("utab", [2 * S, 1], _F32, isOutput=False)
    mk = nc.declare_dram_parameter("mk", [S, 1], _F32, isOutput=False)
    out = nc.declare_dram_parameter("out", [D, S], _F32, isOutput=True)

    Exp = mybir.ActivationFunctionType.Exp
    Copy = mybir.ActivationFunctionType.Copy

    with tile.TileContext(nc) as tc:
        with (
            tc.tile_pool(name="consts", bufs=1) as consts,
            tc.tile_pool(name="wqk_p", bufs=1) as wqk_p,
            tc.tile_pool(name="wv_p", bufs=1) as wv_p,
            tc.tile_pool(name="kqt_p", bufs=1) as kqt_p,
            tc.tile_pool(name="vst_p", bufs=1) as vst_p,
            tc.tile_pool(name="xt_p", bufs=10) as xt_p,
            tc.tile_pool(name="ear_p", bufs=1) as ear_p,
            tc.tile_pool(name="p_p", bufs=5) as p_p,
            tc.tile_pool(name="ctx_p", bufs=1) as ctx_p,
            tc.tile_pool(name="wp_p", bufs=1) as wp_p,
            tc.tile_pool(name="ot_p", bufs=3) as ot_p,
            tc.tile_pool(name="sm_p", bufs=3) as sm_p,
            tc.tile_pool(name="ps", bufs=3, space="PSUM") as ps,
            tc.tile_pool(name="pc", bufs=1, space="PSUM") as pc,
            tc.tile_pool(name="drs", bufs=4, space="DRAM") as drs,
        ):
            # ---- phase-A-critical loads FIRST (sync/HWDGE queue) ----
            xts0 = []
            for dt in range(NDT):
                t = xt_p.tile([128, 1024], _BF, tag="xt", name="xt0")
                nc.sync.dma_start(out=t, in_=xt[dt * 128 : (dt + 1) * 128, 0:1024])
                xts0.append(t)
            wqk_s = []
            for dt in range(NDT):
                t = wqk_p.tile([128, 512], _BF, tag=f"wqk{dt}", name=f"wqk{dt}")
                nc.scalar.dma_start(out=t, in_=wqk[dt * 128 : (dt + 1) * 128, :])
                wqk_s.append(t)
            wv_s = []
            for dt in range(NDT):
                t = wv_p.tile([128, 256], _BF, tag=f"wv{dt}", name=f"wv{dt}")
                nc.gpsimd.dma_start(out=t, in_=wv[dt * 128 : (dt + 1) * 128, :])
                wv_s.append(t)

            # ---- ACT exp table warm-up ----
            dum = consts.tile([1, 1], _F32)
            nc.vector.memset(dum, 0.0)
            nc.scalar.activation(dum, dum, Exp)

            ones1024 = consts.tile([1, 1024], _BF)
            nc.vector.memset(ones1024, 1.0)
            wqkb_s = consts.tile([1, 512], _BF)
            nc.gpsimd.dma_start(out=wqkb_s, in_=wqkb[:, :])
            wvb_s = consts.tile([1, 256], _BF)
            nc.gpsimd.dma_start(out=wvb_s, in_=wvb[:, :])
            mk_s = consts.tile([128, NKT], _F32)
            for kt in range(NKT):
                nc.gpsimd.dma_start(out=mk_s[:, kt : kt + 1],
                                    in_=mk[kt * 128 : (kt + 1) * 128, :])
            utab_s = consts.tile([128, 2 * NKT], _F32)
            for j in range(2):
                for kt in range(NKT):
                    nc.gpsimd.dma_start(
                        out=utab_s[:, j * NKT + kt : j * NKT + kt + 1],
                        in_=utab[j * S + kt * 128 : j * S + (kt + 1) * 128, :])

            wp_s = []
            for hp in range(2):
                t = wp_p.tile([128, D], _BF, tag=f"wp{hp}", name=f"wp{hp}")
                nc.sync.dma_start(out=t, in_=wp[hp * 128 : (hp + 1) * 128, :])
                wp_s.append(t)
            earena = ear_p.tile([128, ETOT * 512], _BF)
            nchunk = 8
            w_ = ETOT * 512 // nchunk
            rem = ETOT * 512 - nchunk * w_
            for c4 in range(nchunk):
                hi = (c4 + 1) * w_ + (rem if c4 == nchunk - 1 else 0)
                nc.sync.dma_start(out=earena[:, c4 * w_ : hi],
                                  in_=etab[:, c4 * w_ : hi])

            # ---- persistent activation tensors ----
            qq = [kqt_p.tile([128, S], _BF, tag=f"qq{p}", name=f"qq{p}") for p in range(2)]
            kk = [kqt_p.tile([128, S], _BF, tag=f"kk{p}", name=f"kk{p}") for p in range(2)]
            vst = [vst_p.tile([128, HPC * 65], _BF, tag=f"vst{kt}", name=f"vst{kt}")
                   for kt in range(NKT)]
            vstR = [vst_p.tile([128, 2 * 65], _BF, tag=f"vstR{kt}", name=f"vstR{kt}")
                    for kt in range(NKT)]
            ctx_s = [ctx_p.tile([128, S], _BF, tag=f"ctx{hp}", name=f"ctx{hp}")
                     for hp in range(2)]

            for kt in range(NKT):
                for h in range(HPC):
                    nc.vector.memset(vst[kt][:, h * 65 + 64 : h * 65 + 65], 1.0)

            # ================= phase A: QKV projections =================
            for stp in range(NW):
                if stp == 0:
                    xts = xts0
                else:
                    xts = []
                    for dt in range(NDT):
                        t = xt_p.tile([128, 1024], _BF, tag="xt", name="xt1")
                        nc.sync.dma_start(
                            out=t, in_=xt[dt * 128 : (dt + 1) * 128,
                                          stp * 1024 : (stp + 1) * 1024])
                        xts.append(t)
                for rt in range(HPC):
                    qk_ps = ps.tile([128, 1024], _F32, tag="ps", name="qk_ps")
                    for c0 in (0, 512):
                        for dt in range(NDT):
                            nc.tensor.matmul(
                                qk_ps[:, c0 : c0 + 512],
                                lhsT=wqk_s[dt][:, rt * 128 : (rt + 1) * 128],
                                rhs=xts[dt][:, c0 : c0 + 512],
                                start=(dt == 0), stop=False,
                            )
                        nc.tensor.matmul(
                            qk_ps[:, c0 : c0 + 512],
                            lhsT=wqkb_s[:, rt * 128 : (rt + 1) * 128],
                            rhs=ones1024[:, c0 : c0 + 512],
                            start=False, stop=True,
                        )
                    dst = (qq if rt % 2 == 0 else kk)[rt // 2]
                    nc.vector.tensor_copy(
                        out=dst[:, stp * 1024 : (stp + 1) * 1024], in_=qk_ps)
                for sub in range(8):
                    kt_i = stp * 8 + sub
                    v_ps = ps.tile([128, 1024], _F32, tag="ps", name="v_ps")
                    for dt in range(NDT):
                        nc.tensor.matmul(
                            v_ps[:, 0:256],
                            lhsT=xts[dt][:, sub * 128 : (sub + 1) * 128],
                            rhs=wv_s[dt],
                            start=(dt == 0), stop=False,
                        )
                    nc.tensor.matmul(
                        v_ps[:, 0:256],
                        lhsT=ones1024[:, 0:128],
                        rhs=wvb_s,
                        start=False, stop=True,
                    )
                    for h in range(HPC):
                        nc.vector.tensor_copy(
                            out=vst[kt_i][:, h * 65 : h * 65 + 64],
                            in_=v_ps[:, h * 64 : (h + 1) * 64])
                    nc.vector.tensor_scalar_mul(
                        vst[kt_i][:, :], vst[kt_i][:, :],
                        mk_s[:, kt_i : kt_i + 1])
                    for j in range(2):
                        nc.vector.tensor_scalar_mul(
                            vstR[kt_i][:, j * 65 : (j + 1) * 65],
                            vst[kt_i][:, (2 + j) * 65 : (3 + j) * 65],
                            utab_s[:, j * NKT + kt_i : j * NKT + kt_i + 1])

            # ================= phase B + C: per q-window =================
            for w in range(NW):
                for h in range(HPC):
                    hp, half = h // 2, h % 2
                    lo, hi = half * 64, half * 64 + 64
                    hr = half * 64
                    kts = BANDS[h][w]
                    ctx_ps = pc.tile([65, 1024], _F32, tag="pc", name="ctx_ps")
                    for i, kt in enumerate(kts):
                        s_ps = ps.tile([128, 1024], _F32, tag="ps", name="s_ps")
                        for c0 in (0, 512):
                            nc.tensor.matmul(
                                s_ps[:, c0 : c0 + 512],
                                lhsT=kk[hp][lo:hi, kt * 128 : (kt + 1) * 128],
                                rhs=qq[hp][lo:hi, w * 1024 + c0 : w * 1024 + c0 + 512],
                                start=True, stop=True,
                            )
                        p_t = p_p.tile([128, 1024], _BF, tag="p", name="p_t")
                        nc.scalar.activation(p_t, s_ps, Exp)
                        for hf in (2 * w, 2 * w + 1):
                            dlt = kt * 128 - hf * 512
                            if h < 2 or dlt > -128:
                                ei = EOFF[h] + EIDX[h][dlt]
                                c0 = (hf - 2 * w) * 512
                                nc.vector.tensor_mul(
                                    p_t[:, c0 : c0 + 512],
                                    p_t[:, c0 : c0 + 512],
                                    earena[:, ei * 512 : (ei + 1) * 512])
                        if h < 2:
                            lhsT = vst[kt][:, h * 65 : (h + 1) * 65]
                        else:
                            lhsT = vstR[kt][:, (h - 2) * 65 : (h - 1) * 65]
                        for c0 in (0, 512):
                            nc.tensor.matmul(
                                ctx_ps[:, c0 : c0 + 512], lhsT=lhsT,
                                rhs=p_t[:, c0 : c0 + 512],
                                start=(i == 0), stop=(i == len(kts) - 1),
                                skip_group_check=True,
                            )
                    # evacuate ctx+rowsum to SBUF, free the psum bank fast
                    cx = sm_p.tile([65, 1024], _F32, tag="cx", name="cx")
                    nc.vector.tensor_copy(out=cx, in_=ctx_ps)
                    # rowsum -> DRAM -> [64,16] lanes -> recip -> DRAM ->
                    # partition-broadcast -> multiply
                    r_d1 = drs.tile([1, 1024], _F32, tag="rd1", name="r_d1")
                    nc.gpsimd.dma_start(out=r_d1, in_=cx[64:65, :])
                    r64 = sm_p.tile([64, 16], _F32, tag="r64", name="r64")
                    nc.gpsimd.dma_start(
                        out=r64, in_=r_d1.rearrange("a (p f) -> (a p) f", p=64))
                    nc.vector.reciprocal(r64, r64)
                    r_d2 = drs.tile([1, 1024], _F32, tag="rd2", name="r_d2")
                    nc.gpsimd.dma_start(
                        out=r_d2.rearrange("a (p f) -> (a p) f", p=64), in_=r64)
                    b_s = sm_p.tile([64, 1024], _F32, tag="bcast", name="b_s")
                    nc.gpsimd.dma_start(out=b_s, in_=r_d2.partition_broadcast(64))
                    nc.vector.tensor_mul(
                        ctx_s[hp][hr : hr + 64, w * 1024 : (w + 1) * 1024],
                        cx[0:64, :], b_s)

                # ---- phase C for this window ----
                for dt in range(NDT):
                    o_ps = ps.tile([128, 1024], _F32, tag="ps", name="o_ps")
                    for c0 in (0, 512):
                        for hp in range(2):
                            nc.tensor.matmul(
                                o_ps[:, c0 : c0 + 512],
                                lhsT=wp_s[hp][:, dt * 128 : (dt + 1) * 128],
                                rhs=ctx_s[hp][:, w * 1024 + c0 : w * 1024 + c0 + 512],
                                start=(hp == 0), stop=(hp == 1),
                            )
                    o_s = ot_p.tile([128, 1024], _F32, tag="ot", name="o_s")
                    nc.vector.tensor_copy(out=o_s, in_=o_ps)
                    nc.sync.dma_start(
                        out=out[dt * 128 : (dt + 1) * 128,
                                w * 1024 : (w + 1) * 1024],
                        in_=o_s)
    nc.finalize()
    return nc


_NC = None


def _get_nc():
    global _NC
    if _NC is None:
        _NC = build_bass()
    return _NC


def _host_inputs(inputs, mask, Wqkv, bqkv, Wproj, bproj):
    x = np.asarray(inputs, np.float32)
    mask = np.asarray(mask)
    Wqkv = np.asarray(Wqkv, np.float32)
    bqkv = np.asarray(bqkv, np.float32)
    Wproj = np.asarray(Wproj, np.float32)

    start = 2.0 ** (-8.0 / H)
    slopes = start ** np.arange(1, H + 1, dtype=np.float64)

    per_g = {}
    ii = np.arange(128, dtype=np.float64)[:, None]
    jj = np.arange(512, dtype=np.float64)[None, :]
    for g in range(4):
        heads = [g, g + 4, g + 8, g + 12]
        wqk = np.empty((D, 2 * HPC * DH), np.float32)
        wqkb = np.empty((1, 2 * HPC * DH), np.float32)
        wv = np.empty((D, HPC * DH), np.float32)
        wvb = np.empty((1, HPC * DH), np.float32)
        wp = np.empty((HPC * DH, D), np.float32)
        etab = np.zeros((128, ETOT * 512), BF16)
        utab = np.empty((2 * S, 1), np.float32)
        for hl, hh in enumerate(heads):
            r0 = hh * 3 * DH
            p, half = hl // 2, hl % 2
            qcol = (2 * p) * 128 + half * 64
            kcol = (2 * p + 1) * 128 + half * 64
            wqk[:, qcol : qcol + 64] = Wqkv[r0 : r0 + DH, :].T * SCALE
            wqk[:, kcol : kcol + 64] = Wqkv[r0 + DH : r0 + 2 * DH, :].T
            wqkb[0, qcol : qcol + 64] = bqkv[r0 : r0 + DH] * SCALE
            wqkb[0, kcol : kcol + 64] = bqkv[r0 + DH : r0 + 2 * DH]
            wv[:, hl * 64 : (hl + 1) * 64] = Wqkv[r0 + 2 * DH : r0 + 3 * DH, :].T
            wvb[0, hl * 64 : (hl + 1) * 64] = bqkv[r0 + 2 * DH : r0 + 3 * DH]
            wp[hl * 64 : (hl + 1) * 64, :] = Wproj[:, hh * DH : (hh + 1) * DH].T
            sl = slopes[hh]
            for dlt in EDELT[hl]:
                ei = EOFF[hl] + EIDX[hl][dlt]
                dd = dlt + ii - jj                    # k - q
                if hl < 2:
                    blk = np.exp(-sl * np.abs(dd))
                else:
                    blk = np.where(dd <= 0, 1.0, np.exp(-2.0 * sl * dd))
                etab[:, ei * 512 : (ei + 1) * 512] = blk
            if hl >= 2:
                kkk = np.arange(S, dtype=np.float64)
                utab[(hl - 2) * S : (hl - 1) * S, 0] = np.exp(sl * (kkk - CENT))
        per_g[g] = dict(wqk=wqk.astype(BF16), wqkb=wqkb.astype(BF16),
                        wv=wv.astype(BF16), wvb=wvb.astype(BF16),
                        wp=wp.astype(BF16), etab=etab, utab=utab)

    in_maps = []
    for c in range(8):
        b, g = c // 4, c % 4
        m = dict(per_g[g])
        m["xt"] = np.ascontiguousarray(x[b].T).astype(BF16)
        m["mk"] = mask[b].astype(np.float32).reshape(S, 1)
        in_maps.append(m)
    return in_maps


def kernel(inputs, mask, Wqkv, bqkv, Wproj, bproj, _want_trace=False):
    nc = _get_nc()
    in_maps = _host_inputs(inputs, mask, Wqkv, bqkv, Wproj, bproj)
    res = run_bass_kernel_spmd(nc, in_maps, core_ids=list(range(8)),
                               trace=_want_trace)
    outs = res.results
    out = np.zeros((B, S, D), np.float32)
    for c in range(8):
        out[c // 4] += outs[c]["out"].T
    out += np.asarray(bproj, np.float32)
    if _want_trace:
        kernel.last_result = res
    return out


# revision 23
# speedup vs baseline: 1.2028x; 1.2028x over previous
"""AltAttention (B=2,S=2048,D=1024,H=16, ALiBi + key-mask) on 8 TRN2 cores.

Sharding: core c = (b = c//4, head-group g = c%4 -> heads {g, g+4, g+8, g+12}).
Each core computes QKV for its 4 heads, attention, and a partial output
projection (row-split Wproj).  Host sums the 4 partials per batch and adds
bproj.

On-chip layout is fully transposed: scores S^T=[k,q], context ctx^T=[dh,q],
output out^T=[dout,q].  All matmuls bf16 (fp32 PSUM accumulate), attention
and projection use N=1024 moving operands.

ALiBi:
 - banding: tiles where exp(-slope|k-q|) < e^-15 everywhere are skipped.
   Band profile per local head slot = max cutoff over the 4 interleaved
   head sets, so the graph is SPMD-identical across cores.
 - local heads 0,1 (steep): P = exp(S+mask) * E, E = exp(-slope|k-q|)
   host-precomputed per diagonal offset (bf16, deduplicated).
 - local heads 2,3 (shallow): exp(-slope|k-q|) = [u(k)*v(q)] * R where
   u(k)=e^{slope(k-1024)} is folded into a scaled copy of [V|1], v(q)
   cancels in softmax normalization, and R = 1 for k<=q,
   exp(-2 slope (k-q)) for k>q is a (sparser) E-table multiply.
"""

import sys

for _p in ("/opt/trn_rl_repo", "/opt/pypackages"):
    if _p not in sys.path:
        sys.path.insert(0, _p)

import numpy as np
import ml_dtypes

import concourse.bass as bass
from concourse import bacc
import concourse.mybir as mybir
import concourse.tile as tile
from concourse.bass_utils import run_bass_kernel_spmd

BF16 = ml_dtypes.bfloat16

B, S, D, H = 2, 2048, 1024, 16
DH = D // H
HPC = 4
SCALE = D ** -0.5
NKT = S // 128       # 16
NW = S // 1024       # 2 q-windows of 1024
NDT = D // 128       # 8
CENT = 1024

CUTS = [60, 240, 960, 99999]


def _band(hl, w):
    cut = CUTS[hl]
    q0 = w * 1024
    lo, hi = max(0, q0 - cut), min(S, q0 + 1024 + cut)
    return [kt for kt in range(NKT) if kt * 128 < hi and (kt + 1) * 128 > lo]


BANDS = [[_band(hl, w) for w in range(NW)] for hl in range(4)]

# E multiply works at [128,512] half-window granularity.
# delta = kt*128 - half*512 over all (kt, half) pairs of the banded windows.
EDELT = {}
for hl in range(4):
    ds = set()
    for w in range(NW):
        for kt in BANDS[hl][w]:
            for half in (2 * w, 2 * w + 1):
                dlt = kt * 128 - half * 512
                if hl < 2 or dlt > -128:
                    ds.add(dlt)
    EDELT[hl] = sorted(ds)
EIDX = {hl: {d: i for i, d in enumerate(EDELT[hl])} for hl in range(4)}
ESLOT = [len(EDELT[hl]) for hl in range(4)]
EOFF = [0, ESLOT[0], ESLOT[0] + ESLOT[1], ESLOT[0] + ESLOT[1] + ESLOT[2]]
ETOT = sum(ESLOT)

_F32 = mybir.dt.float32
_BF = mybir.dt.bfloat16


def build_bass():
    nc = bacc.Bacc(None, target_bir_lowering=False)
    xt = nc.declare_dram_parameter("xt", [D, S], _BF, isOutput=False)
    wqk = nc.declare_dram_parameter("wqk", [D, 2 * HPC * DH], _BF, isOutput=False)
    wqkb = nc.declare_dram_parameter("wqkb", [1, 2 * HPC * DH], _BF, isOutput=False)
    wv = nc.declare_dram_parameter("wv", [D, HPC * DH], _BF, isOutput=False)
    wvb = nc.declare_dram_parameter("wvb", [1, HPC * DH], _BF, isOutput=False)
    wp = nc.declare_dram_parameter("wp", [HPC * DH, D], _BF, isOutput=False)
    etab = nc.declare_dram_parameter("etab", [128, ETOT * 512], _BF, isOutput=False)
    utab = nc.declare_dram_parameter("utab", [2 * S, 1], _F32, isOutput=False)
    mk = nc.declare_dram_parameter("mk", [S, 1], _F32, isOutput=False)
    out = nc.declare_dram_parameter("out", [D, S], _F32, isOutput=True)

    Exp = mybir.ActivationFunctionType.Exp
    Copy = mybir.ActivationFunctionType.Copy

    with tile.TileContext(nc) as tc:
        with (
            tc.tile_pool(name="consts", bufs=1) as consts,
            tc.tile_pool(name="wqk_p", bufs=1) as wqk_p,
            tc.tile_pool(name="wv_p", bufs=1) as wv_p,
            tc.tile_pool(name="kqt_p", bufs=1) as kqt_p,
            tc.tile_pool(name="vst_p", bufs=1) as vst_p,
            tc.tile_pool(name="xt_p", bufs=10) as xt_p,
            tc.tile_pool(name="ear_p", bufs=1) as ear_p,
            tc.tile_pool(name="p_p", bufs=5) as p_p,
            tc.tile_pool(name="ctx_p", bufs=1) as ctx_p,
            tc.tile_pool(name="wp_p", bufs=1) as wp_p,
            tc.tile_pool(name="ot_p", bufs=3) as ot_p,
            tc.tile_pool(name="sm_p", bufs=3) as sm_p,
            tc.tile_pool(name="ps", bufs=3, space="PSUM") as ps,
            tc.tile_pool(name="pc", bufs=1, space="PSUM") as pc,
            tc.tile_pool(name="drs", bufs=4, space="DRAM") as drs,
        ):
            # ---- phase-A-critical loads FIRST (sync/HWDGE queue) ----
            xts0 = []
            for dt in range(NDT):
                t = xt_p.tile([128, 1024], _BF, tag="xt", name="xt0")
                nc.sync.dma_start(out=t, in_=xt[dt * 128 : (dt + 1) * 128, 0:1024])
                xts0.append(t)
            wqk_s = []
            for dt in range(NDT):
                t = wqk_p.tile([128, 512], _BF, tag=f"wqk{dt}", name=f"wqk{dt}")
                nc.scalar.dma_start(out=t, in_=wqk[dt * 128 : (dt + 1) * 128, :])
                wqk_s.append(t)
            wv_s = []
            for dt in range(NDT):
                t = wv_p.tile([128, 256], _BF, tag=f"wv{dt}", name=f"wv{dt}")
                nc.gpsimd.dma_start(out=t, in_=wv[dt * 128 : (dt + 1) * 128, :])
                wv_s.append(t)

            # ---- ACT exp table warm-up ----
            dum = consts.tile([1, 1], _F32)
            nc.vector.memset(dum, 0.0)
            nc.scalar.activation(dum, dum, Exp)

            ones1024 = consts.tile([1, 1024], _BF)
            nc.vector.memset(ones1024, 1.0)
            wqkb_s = consts.tile([1, 512], _BF)
            nc.gpsimd.dma_start(out=wqkb_s, in_=wqkb[:, :])
            wvb_s = consts.tile([1, 256], _BF)
            nc.gpsimd.dma_start(out=wvb_s, in_=wvb[:, :])
            mk_s = consts.tile([128, NKT], _F32)
            for kt in range(NKT):
                nc.gpsimd.dma_start(out=mk_s[:, kt : kt + 1],
                                    in_=mk[kt * 128 : (kt + 1) * 128, :])
            utab_s = consts.tile([128, 2 * NKT], _F32)
            for j in range(2):
                for kt in range(NKT):
                    nc.gpsimd.dma_start(
                        out=utab_s[:, j * NKT + kt : j * NKT + kt + 1],
                        in_=utab[j * S + kt * 128 : j * S + (kt + 1) * 128, :])

            wp_s = []
            for hp in range(2):
                t = wp_p.tile([128, D], _BF, tag=f"wp{hp}", name=f"wp{hp}")
                nc.sync.dma_start(out=t, in_=wp[hp * 128 : (hp + 1) * 128, :])
                wp_s.append(t)
            earena = ear_p.tile([128, ETOT * 512], _BF)
            nchunk = 8
            w_ = ETOT * 512 // nchunk
            rem = ETOT * 512 - nchunk * w_
            for c4 in range(nchunk):
                hi = (c4 + 1) * w_ + (rem if c4 == nchunk - 1 else 0)
                nc.sync.dma_start(out=earena[:, c4 * w_ : hi],
                                  in_=etab[:, c4 * w_ : hi])

            # ---- persistent activation tensors ----
            qq = [kqt_p.tile([128, S], _BF, tag=f"qq{p}", name=f"qq{p}") for p in range(2)]
            kk = [kqt_p.tile([128, S], _BF, tag=f"kk{p}", name=f"kk{p}") for p in range(2)]
            vst = [vst_p.tile([128, HPC * 65], _BF, tag=f"vst{kt}", name=f"vst{kt}")
                   for kt in range(NKT)]
            vstR = [vst_p.tile([128, 2 * 65], _BF, tag=f"vstR{kt}", name=f"vstR{kt}")
                    for kt in range(NKT)]
            ctx_s = [ctx_p.tile([128, S], _BF, tag=f"ctx{hp}", name=f"ctx{hp}")
                     for hp in range(2)]

            for kt in range(NKT):
                for h in range(HPC):
                    nc.vector.memset(vst[kt][:, h * 65 + 64 : h * 65 + 65], 1.0)

            # ================= phase A: QKV projections =================
            for stp in range(NW):
                if stp == 0:
                    xts = xts0
                else:
                    xts = []
                    for dt in range(NDT):
                        t = xt_p.tile([128, 1024], _BF, tag="xt", name="xt1")
                        nc.sync.dma_start(
                            out=t, in_=xt[dt * 128 : (dt + 1) * 128,
                                          stp * 1024 : (stp + 1) * 1024])
                        xts.append(t)
                for rt in range(HPC):
                    qk_ps = ps.tile([128, 1024], _F32, tag="ps", name="qk_ps")
                    for c0 in (0, 512):
                        for dt in range(NDT):
                            nc.tensor.matmul(
                                qk_ps[:, c0 : c0 + 512],
                                lhsT=wqk_s[dt][:, rt * 128 : (rt + 1) * 128],
                                rhs=xts[dt][:, c0 : c0 + 512],
                                start=(dt == 0), stop=False,
                            )
                        nc.tensor.matmul(
                            qk_ps[:, c0 : c0 + 512],
                            lhsT=wqkb_s[:, rt * 128 : (rt + 1) * 128],
                            rhs=ones1024[:, c0 : c0 + 512],
                            start=False, stop=True,
                        )
                    dst = (qq if rt % 2 == 0 else kk)[rt // 2]
                    nc.vector.tensor_copy(
                        out=dst[:, stp * 1024 : (stp + 1) * 1024], in_=qk_ps)
                for sub in range(8):
                    kt_i = stp * 8 + sub
                    v_ps = ps.tile([128, 1024], _F32, tag="ps", name="v_ps")
                    for dt in range(NDT):
                        nc.tensor.matmul(
                            v_ps[:, 0:256],
                            lhsT=xts[dt][:, sub * 128 : (sub + 1) * 128],
                            rhs=wv_s[dt],
                            start=(dt == 0), stop=False,
                        )
                    nc.tensor.matmul(
                        v_ps[:, 0:256],
                        lhsT=ones1024[:, 0:128],
                        rhs=wvb_s,
                        start=False, stop=True,
                    )
                    for h in range(HPC):
                        nc.vector.tensor_copy(
                            out=vst[kt_i][:, h * 65 : h * 65 + 64],
                            in_=v_ps[:, h * 64 : (h + 1) * 64])
                    nc.vector.tensor_scalar_mul(
                        vst[kt_i][:, :], vst[kt_i][:, :],
                        mk_s[:, kt_i : kt_i + 1])
                    for j in range(2):
                        nc.vector.tensor_scalar_mul(
                            vstR[kt_i][:, j * 65 : (j + 1) * 65],
                            vst[kt_i][:, (2 + j) * 65 : (3 + j) * 65],
                            utab_s[:, j * NKT + kt_i : j * NKT + kt_i + 1])

            # ================= phase B: attention =================
            for h in range(HPC):
                hp, half = h // 2, h % 2
                lo, hi = half * 64, half * 64 + 64
                hr = half * 64
                for w in range(NW):
                    kts = BANDS[h][w]
                    ctx_ps = pc.tile([65, 1024], _F32, tag="pc", name="ctx_ps")
                    for i, kt in enumerate(kts):
                        s_ps = ps.tile([128, 1024], _F32, tag="ps", name="s_ps")
                        for c0 in (0, 512):
                            nc.tensor.matmul(
                                s_ps[:, c0 : c0 + 512],
                                lhsT=kk[hp][lo:hi, kt * 128 : (kt + 1) * 128],
                                rhs=qq[hp][lo:hi, w * 1024 + c0 : w * 1024 + c0 + 512],
                                start=True, stop=True,
                            )
                        p_t = p_p.tile([128, 1024], _BF, tag="p", name="p_t")
                        nc.scalar.activation(p_t, s_ps, Exp)
                        for hf in (2 * w, 2 * w + 1):
                            dlt = kt * 128 - hf * 512
                            if h < 2 or dlt > -128:
                                ei = EOFF[h] + EIDX[h][dlt]
                                c0 = (hf - 2 * w) * 512
                                nc.vector.tensor_mul(
                                    p_t[:, c0 : c0 + 512],
                                    p_t[:, c0 : c0 + 512],
                                    earena[:, ei * 512 : (ei + 1) * 512])
                        if h < 2:
                            lhsT = vst[kt][:, h * 65 : (h + 1) * 65]
                        else:
                            lhsT = vstR[kt][:, (h - 2) * 65 : (h - 1) * 65]
                        for c0 in (0, 512):
                            nc.tensor.matmul(
                                ctx_ps[:, c0 : c0 + 512], lhsT=lhsT,
                                rhs=p_t[:, c0 : c0 + 512],
                                start=(i == 0), stop=(i == len(kts) - 1),
                                skip_group_check=True,
                            )
                    # evacuate ctx+rowsum to SBUF, free the psum bank fast
                    cx = sm_p.tile([65, 1024], _F32, tag="cx", name="cx")
                    nc.vector.tensor_copy(out=cx, in_=ctx_ps)
                    r_d1 = drs.tile([1, 1024], _F32, tag="rd1", name="r_d1")
                    nc.gpsimd.dma_start(out=r_d1, in_=cx[64:65, :])
                    r64 = sm_p.tile([64, 16], _F32, tag="r64", name="r64")
                    nc.gpsimd.dma_start(
                        out=r64, in_=r_d1.rearrange("a (p f) -> (a p) f", p=64))
                    nc.vector.reciprocal(r64, r64)
                    r_d2 = drs.tile([1, 1024], _F32, tag="rd2", name="r_d2")
                    nc.gpsimd.dma_start(
                        out=r_d2.rearrange("a (p f) -> (a p) f", p=64), in_=r64)
                    b_s = sm_p.tile([64, 1024], _F32, tag="bcast", name="b_s")
                    nc.gpsimd.dma_start(out=b_s, in_=r_d2.partition_broadcast(64))
                    nc.vector.tensor_mul(
                        ctx_s[hp][hr : hr + 64, w * 1024 : (w + 1) * 1024],
                        cx[0:64, :], b_s)

            # ================= phase C: output projection =================
            for w in range(NW):
                for dt in range(NDT):
                    o_ps = ps.tile([128, 1024], _F32, tag="ps", name="o_ps")
                    for c0 in (0, 512):
                        for hp in range(2):
                            nc.tensor.matmul(
                                o_ps[:, c0 : c0 + 512],
                                lhsT=wp_s[hp][:, dt * 128 : (dt + 1) * 128],
                                rhs=ctx_s[hp][:, w * 1024 + c0 : w * 1024 + c0 + 512],
                                start=(hp == 0), stop=(hp == 1),
                            )
                    o_s = ot_p.tile([128, 1024], _F32, tag="ot", name="o_s")
                    nc.vector.tensor_copy(out=o_s, in_=o_ps)
                    nc.sync.dma_start(
                        out=out[dt * 128 : (dt + 1) * 128,
                                w * 1024 : (w + 1) * 1024],
                        in_=o_s)
    nc.finalize()
    return nc


_NC = None


def _get_nc():
    global _NC
    if _NC is None:
        _NC = build_bass()
    return _NC


def _host_inputs(inputs, mask, Wqkv, bqkv, Wproj, bproj):
    x = np.asarray(inputs, np.float32)
    mask = np.asarray(mask)
    Wqkv = np.asarray(Wqkv, np.float32)
    bqkv = np.asarray(bqkv, np.float32)
    Wproj = np.asarray(Wproj, np.float32)

    start = 2.0 ** (-8.0 / H)
    slopes = start ** np.arange(1, H + 1, dtype=np.float64)

    per_g = {}
    ii = np.arange(128, dtype=np.float64)[:, None]
    jj = np.arange(512, dtype=np.float64)[None, :]
    for g in range(4):
        heads = [g, g + 4, g + 8, g + 12]
        wqk = np.empty((D, 2 * HPC * DH), np.float32)
        wqkb = np.empty((1, 2 * HPC * DH), np.float32)
        wv = np.empty((D, HPC * DH), np.float32)
        wvb = np.empty((1, HPC * DH), np.float32)
        wp = np.empty((HPC * DH, D), np.float32)
        etab = np.zeros((128, ETOT * 512), BF16)
        utab = np.empty((2 * S, 1), np.float32)
        for hl, hh in enumerate(heads):
            r0 = hh * 3 * DH
            p, half = hl // 2, hl % 2
            qcol = (2 * p) * 128 + half * 64
            kcol = (2 * p + 1) * 128 + half * 64
            wqk[:, qcol : qcol + 64] = Wqkv[r0 : r0 + DH, :].T * SCALE
            wqk[:, kcol : kcol + 64] = Wqkv[r0 + DH : r0 + 2 * DH, :].T
            wqkb[0, qcol : qcol + 64] = bqkv[r0 : r0 + DH] * SCALE
            wqkb[0, kcol : kcol + 64] = bqkv[r0 + DH : r0 + 2 * DH]
            wv[:, hl * 64 : (hl + 1) * 64] = Wqkv[r0 + 2 * DH : r0 + 3 * DH, :].T
            wvb[0, hl * 64 : (hl + 1) * 64] = bqkv[r0 + 2 * DH : r0 + 3 * DH]
            wp[hl * 64 : (hl + 1) * 64, :] = Wproj[:, hh * DH : (hh + 1) * DH].T
            sl = slopes[hh]
            for dlt in EDELT[hl]:
                ei = EOFF[hl] + EIDX[hl][dlt]
                dd = dlt + ii - jj                    # k - q
                if hl < 2:
                    blk = np.exp(-sl * np.abs(dd))
                else:
                    blk = np.where(dd <= 0, 1.0, np.exp(-2.0 * sl * dd))
                etab[:, ei * 512 : (ei + 1) * 512] = blk
            if hl >= 2:
                kkk = np.arange(S, dtype=np.float64)
                utab[(hl - 2) * S : (hl - 1) * S, 0] = np.exp(sl * (kkk - CENT))
        per_g[g] = dict(wqk=wqk.astype(BF16), wqkb=wqkb.astype(BF16),
                        wv=wv.astype(BF16), wvb=wvb.astype(BF16),
                        wp=wp.astype(BF16), etab=etab, utab=utab)

    in_maps = []
    for c in range(8):
        b, g = c // 4, c % 4
        m = dict(per_g[g])
        m["xt"] = np.ascontiguousarray(x[b].T).astype(BF16)
        m["mk"] = mask[b].astype(np.float32).reshape(S, 1)
        in_maps.append(m)
    return in_maps


def kernel(inputs, mask, Wqkv, bqkv, Wproj, bproj, _want_trace=False):
    nc = _get_nc()
    in_maps = _host_inputs(inputs, mask, Wqkv, bqkv, Wproj, bproj)
    res = run_bass_kernel_spmd(nc, in_maps, core_ids=list(range(8)),
                               trace=_want_trace)
    outs = res.results
    out = np.zeros((B, S, D), np.float32)
    for c in range(8):
        out[c // 4] += outs[c]["out"].T
    out += np.asarray(bproj, np.float32)
    if _want_trace:
        kernel.last_result = res
    return out
